# revision 20
# baseline (speedup 1.0000x reference)
"""Trainium2 Bass kernel for nn_Attention_32049045963483 (sparse_attention).

Math collapse (validated vs reference at ~4e-4 l2 rel err):
  - qkv 1x1 conv folds into the 11x11/stride-8 down-convs HOST-SIDE:
      conv(W1 @ f, wq) == conv(f, w_eff)   (weight preprocessing)
  - 64x nearest upsample + softmax == softmax of the low-res score map;
    every output row depends only on x = i//64.
  - v enters only through 64-wide block sums: vbar = Wv @ fbar,
      fbar[d,J] = sum_y f[d,J,y]  (v never materializes)
  - out[c,x,y] = (sum_J e[J,x] vbar[J,c]) / (64 sum_J e[J,x]), broadcast on y.
  - exp via the tanh table (exact identity): e^x = 2/(1 - tanh(x/2)) - 1.
    tanh shares an ACT table with gelu's load set, so no mid-kernel 1.3us
    table reload (Exp forces one: the ACT table slots reload on every
    func-set switch).  Safe: scale*dots in [-0.5, 2.7].

Sharding: head-parallel over 8 cores; core i takes head i.  Each core reads
full f (the down-convs mix all 64 channels).

Stage 1 packs TWO kx taps per matmul into the full 128-partition contraction.
f2E is a phase-major permutation of padded f: rows 0..63 hold EVEN columns
(c=2m) laid out as [r, m%4, m//4], rows 64..127 the ODD columns (the +1 tap).
This gives a contiguous 8-element inner dim (fp16 needs contiguity for
1 col/cycle; strided-8 fp16 measured 2x slower) and stores each element once.
fbar drops out as the full 36-slot row sums of BOTH partition halves (pads
are zero); Wv is stacked twice in the vbar contraction so no half-add needed.

Stage 2 folds the conv biases in as a 12th accumulating matmul (identity
stationary x host-packed bias pattern), so ONE fused gelu covers q and k.

All inputs ride in ONE dram blob with 2 DMA pieces per HWDGE ring
(partition-split; more pieces choke shared descriptor engine 79).  The
output is written x-major [64, 512] (2KB contiguous rows -> big DMA packets);
the host transposes to channel-major.  A ~5.4us junk-matmul warm-up during
the DMA wait ramps the PE clock (0.83 -> 0.43 ns/col) and must stay
gap-free into stage 1: a ~1us idle drops the clock back (measured).
"""

import numpy as np

N_CORES = 8
SCALE = 8.0 ** -0.5  # dim_head ** -0.5

# blob column map (fp16 elements)
C_WEQ = 0
C_F2A = 528
C_WEK = 1752
C_F2B = 2280
C_WS = 3468
C_BIAS = 3556
C_W2 = 3684
C_TOT = 3692

_CACHE = {}

LAST_RESULTS = None  # BassKernelResults of the most recent run (for test harness)


def _dep(after, before, sync=False):
    from concourse.tile import add_dep_helper

    a = getattr(after, "ins", after)
    b = getattr(before, "ins", before)
    add_dep_helper(a, b, sync=sync, reason="pin order")


def _build_nc():
    from contextlib import ExitStack

    import concourse.bacc as bacc
    import concourse.bass as bass
    import concourse.mybir as mybir
    import concourse.tile as tile

    f32 = mybir.dt.float32
    f16 = mybir.dt.float16
    X = mybir.AxisListType.X
    AF = mybir.ActivationFunctionType
    ALU = mybir.AluOpType

    nc = bacc.Bacc("TRN2", target_bir_lowering=False)

    blob_d = nc.dram_tensor("blob", [128, C_TOT], f16, kind="ExternalInput")
    out_d = nc.dram_tensor("out", [64, 512], f16, kind="ExternalOutput")

    with tile.TileContext(nc) as tc:
        with ExitStack() as ctx:
            sb = ctx.enter_context(tc.tile_pool(name="sb", bufs=1))
            ps = ctx.enter_context(tc.tile_pool(name="ps", bufs=1, space="PSUM"))

            blob_t = sb.tile([128, C_TOT], f16)
            warm_t = sb.tile([128, 384], f16)
            fb16_t = sb.tile([128, 64], f16)
            s_t = sb.tile([88, 67 * 16], f16)
            qk_t = sb.tile([8, 128], f16)
            th_t = sb.tile([64, 64], f32)
            u_t = sb.tile([64, 64], f32)
            e_t = sb.tile([64, 64], f16)
            fb_t = sb.tile([128, 64], f32)
            vaug_t = sb.tile([64, 9], f16)
            rs_t = sb.tile([64, 1], f32)
            T_t = sb.tile([64, 8 * 64], f16)
            scr_t = sb.tile([1, 1], f32)
            scr2_t = sb.tile([1, 1], f32)

            # --- input DMAs: 2 pieces per ring (partition-split), then w2
            d_p1s = nc.sync.dma_start(
                out=blob_t[0:64, 0:C_WEK], in_=blob_d[0:64, 0:C_WEK]
            )
            d_p1c = nc.scalar.dma_start(
                out=blob_t[64:128, 0:C_WEK], in_=blob_d[64:128, 0:C_WEK]
            )
            d_p2s = nc.sync.dma_start(
                out=blob_t[0:64, C_WEK:C_TOT], in_=blob_d[0:64, C_WEK:C_TOT]
            )
            d_p2c = nc.scalar.dma_start(
                out=blob_t[64:128, C_WEK:C_TOT], in_=blob_d[64:128, C_WEK:C_TOT]
            )

            # constants + ACT warm-up: dummy Gelu forces its table load early;
            # the load pass hoists tanh's set load next to it (measured).
            nc.vector.memset(scr_t, 0.0)
            nc.vector.memset(vaug_t[:, 8:9], 64.0)
            dg = nc.scalar.activation(out=scr2_t, in_=scr_t, func=AF.Gelu)
            dt = nc.scalar.activation(out=scr2_t, in_=scr_t, func=AF.Tanh)
            _dep(dg, d_p2c)
            _dep(dt, dg)

            # PE warm-up: ~5us of dummy matmuls during the DMA wait ramp the
            # Tensor engine clock.  One stationary, 18 moving passes, junk
            # PSUM.  Ends right as piece1's completion semaphore fires; any
            # idle gap here drops the clock back (measured on HW).
            nc.vector.memset(warm_t, 0.0)
            ps_w = ps.tile([128, 256], f32, tag="H")
            for w in range(18):
                nc.tensor.matmul(
                    ps_w, warm_t[:, 0:128], warm_t[:, 128:384],
                    start=(w == 0), stop=(w == 17),
                )

            f23A = blob_t[:, C_F2A:C_WEK].rearrange("p (r s) -> p r s", s=36)
            f23B = blob_t[:, C_F2B:C_WS].rearrange("p (r s) -> p r s", s=36)
            ws_v = blob_t[0:88, C_WS:C_BIAS]
            bm_v = blob_t[0:8, C_BIAS:C_W2]
            s3 = s_t.rearrange("p (r c16) -> p r c16", c16=16)
            wv2_v = blob_t[:, C_W2:C_TOT]

            # --- stage 1: 2 r-chunks x 2 convs x 6 kx-pairs, fp16, 128-deep
            ps_Aq = ps.tile([88, 34 * 8], f32, tag="A")
            ps_Ak = ps.tile([88, 34 * 8], f32, tag="B")
            ps_Bq = ps.tile([88, 33 * 8], f32, tag="C")
            ps_Bk = ps.tile([88, 33 * 8], f32, tag="D")

            def s1(f23c, pst, wbase):
                for g in range(6):
                    base = (g % 4) * 9 + (g // 4)
                    nc.tensor.matmul(
                        pst,
                        blob_t[:, wbase + g * 88 : wbase + g * 88 + 88],
                        f23c[:, :, base : base + 8],
                        start=(g == 0),
                        stop=(g == 5),
                    )

            def s1cast(eng, pin, out):
                if eng == "v":
                    nc.vector.tensor_copy(out=out, in_=pin)
                else:
                    nc.scalar.copy(out=out, in_=pin)

            def ps3(pst):
                return pst.rearrange("p (r ox) -> p r ox", ox=8)

            # DVE order: redA, castqA, redB, castqB, castkB-h1, vaug, chain
            # ACT order: dummies, castkA, castkB-h2, gelu, tanh
            s1(f23A, ps_Aq, C_WEQ)
            nc.vector.reduce_sum(out=fb_t[:, 0:32], in_=f23A[:, 2:34, :], axis=X)
            s1cast("v", ps3(ps_Aq), s3[:, 0:34, 0:8])
            s1(f23A, ps_Ak, C_WEK)
            s1cast("a", ps3(ps_Ak), s3[:, 0:34, 8:16])
            s1(f23B, ps_Bq, C_WEQ)
            nc.vector.reduce_sum(out=fb_t[:, 32:64], in_=f23B[:, 0:32, :], axis=X)
            s1cast("v", ps3(ps_Bq), s3[:, 34:67, 0:8])
            s1(f23B, ps_Bk, C_WEK)
            s1cast("a", ps3(ps_Bk), s3[:, 34:67, 8:16])

            # --- stage 2: 11 ky-selection matmuls (q+k fused, 128 cols)
            # + 1 bias matmul (identity stationary x host bias pattern)
            psc = ps.tile([8, 128], f32, tag="E")
            for ky in range(11):
                a = s_t[:, ky * 16 : 1072]
                mv = bass.AP(
                    tensor=a.tensor,
                    offset=a.offset,
                    ap=[list(a.ap[0]), [128, 8], [1, 16]],
                )
                nc.tensor.matmul(
                    psc,
                    ws_v[:, ky * 8 : ky * 8 + 8],
                    mv,
                    start=(ky == 0),
                    stop=False,
                )
            nc.tensor.matmul(
                psc, ws_v[0:8, 0:8], bm_v, start=False, stop=True
            )

            # --- vbar: psv[J, c] = sum over 128 rows of fb * [Wv.T; Wv.T]
            psv = ps.tile([64, 8], f32, tag="G")
            nc.vector.tensor_copy(out=fb16_t, in_=fb_t)
            nc.tensor.matmul(psv, fb16_t, wv2_v, start=True, stop=True)
            nc.vector.tensor_copy(out=vaug_t[:, 0:8], in_=psv)

            # --- ONE fused gelu (exact, table) over q|k -> fp16 qk,
            # de-interleaving (oy, cv, ox) -> [q cols 0:64 | k cols 64:128]
            # so the dots matmul gets contiguous operands.
            pin = psc[:, 0:128]
            g_in = bass.AP(
                tensor=pin.tensor, offset=pin.offset,
                ap=[list(pin.ap[0]), [16, 8], [8, 2], [1, 8]],
            )
            qo = qk_t[:, 0:128]
            g_out = bass.AP(
                tensor=qo.tensor, offset=qo.offset,
                ap=[list(qo.ap[0]), [8, 8], [64, 2], [1, 8]],
            )
            nc.scalar.activation(out=g_out, in_=g_in, func=AF.Gelu, scale=1.0)

            # --- dots^T[J, I] (fp16) then e via tanh identity + fast recip
            psd = ps.tile([64, 64], f32, tag="F")
            nc.tensor.matmul(
                psd, qk_t[:, 64:128], qk_t[:, 0:64], start=True, stop=True
            )
            nc.scalar.activation(out=th_t, in_=psd, func=AF.Tanh, scale=SCALE * 0.5)
            nc.vector.tensor_scalar(
                out=u_t, in0=th_t, scalar1=-1.0, scalar2=1.0,
                op0=ALU.mult, op1=ALU.add,
            )
            nc.vector.reciprocal_approx_fast(out=th_t, in_=u_t)
            nc.vector.tensor_scalar(
                out=e_t, in0=th_t, scalar1=2.0, scalar2=1.0,
                op0=ALU.mult, op1=ALU.subtract,
            )

            # --- out_u[I, 0:8] = sum_J e[J,I] vaug[J,:]; col 8 = 64*sum_J e
            pso = ps.tile([64, 9], f32, tag="H")
            nc.tensor.matmul(pso, e_t, vaug_t, start=True, stop=True)
            nc.vector.reciprocal_approx_fast(out=rs_t, in_=pso[:, 8:9])

            # --- normalize + broadcast along y, split by channel halves so
            # each half's DMA (own ring) launches off its own DVE op.
            # out is x-major: out[x, (c, y)] - 1KB contiguous dest rows.
            T3 = T_t.rearrange("p (c y) -> p c y", y=64)
            for h, dma_eng in ((0, nc.sync), (1, nc.scalar)):
                ao = pso[:, 4 * h : 4 * h + 4]
                o_b = bass.AP(
                    tensor=ao.tensor, offset=ao.offset,
                    ap=[list(ao.ap[0]), [1, 4], [0, 64]],
                )
                nc.vector.tensor_scalar_mul(T3[:, 4 * h : 4 * h + 4, :], o_b, rs_t)
                dma_eng.dma_start(
                    out=out_d[:, 256 * h : 256 * h + 256],
                    in_=T3[:, 4 * h : 4 * h + 4, :],
                )

    nc.finalize()
    return nc


def _get_nc():
    if "nc" not in _CACHE:
        _CACHE["nc"] = _build_nc()
    return _CACHE["nc"]


def kernel(**inputs):
    global LAST_RESULTS
    from concourse.bass_utils import run_bass_kernel_spmd

    f = np.ascontiguousarray(inputs["f"], np.float32)
    w_qkv = np.ascontiguousarray(inputs["w_qkv"], np.float32)[:, :, 0, 0]  # [192,64]
    wq = np.ascontiguousarray(inputs["wq"], np.float32)
    wk = np.ascontiguousarray(inputs["wk"], np.float32)
    bq = np.ascontiguousarray(inputs["bq"], np.float32)
    bk = np.ascontiguousarray(inputs["bk"], np.float32)

    W1q, W1k, Wv = w_qkv[0:64], w_qkv[64:128], w_qkv[128:192]

    # f2E phase-major permutation: slot s = (m%4)*9 + m//4 holds column 2m
    # (rows 0..63) / column 2m+1 (rows 64..127) of the padded f.
    fpad = np.zeros((64, 68, 68), np.float32)
    fpad[:, 2:66, 2:66] = f[0]
    f2 = np.zeros((128, 67, 36), np.float32)
    for m in range(34):
        s = (m % 4) * 9 + m // 4
        f2[0:64, :, s] = fpad[:, 0:67, 2 * m]
        if 2 * m + 1 <= 67:
            f2[64:128, :, s] = fpad[:, 0:67, 2 * m + 1]
    f2 = f2.astype(np.float16)

    eye88 = np.eye(88, dtype=np.float16)

    in_maps = []
    for i in range(N_CORES):
        sl = slice(8 * i, 8 * i + 8)
        # w_eff[d, kx, ky, oc] = sum_ic wq[oc,ic,ky,kx] W1[ic,d]
        wEq = np.einsum("oiyx,id->dxyo", wq[sl], W1q)
        wEk = np.einsum("oiyx,id->dxyo", wk[sl], W1k)
        wE = np.zeros((128, 12, 88), np.float16)
        for g in range(6):
            wE[0:64, g] = wEq[:, 2 * g].reshape(64, 88)
            wE[0:64, 6 + g] = wEk[:, 2 * g].reshape(64, 88)
            if 2 * g + 1 <= 10:
                wE[64:128, g] = wEq[:, 2 * g + 1].reshape(64, 88)
                wE[64:128, 6 + g] = wEk[:, 2 * g + 1].reshape(64, 88)
        blob = np.zeros((128, C_TOT), np.float16)
        blob[:, C_WEQ:C_F2A] = wE[:, 0:6].reshape(128, 528)
        blob[:, C_F2A:C_WEK] = f2[:, 0:34, :].reshape(128, 1224)
        blob[:, C_WEK:C_F2B] = wE[:, 6:12].reshape(128, 528)
        blob[:, C_F2B:C_WS] = f2[:, 34:67, :].reshape(128, 1188)
        blob[0:88, C_WS:C_BIAS] = eye88
        bm = np.zeros((8, 8, 2, 8), np.float16)
        bm[:, :, 0, :] = bq[sl].astype(np.float16)[:, None, None]
        bm[:, :, 1, :] = bk[sl].astype(np.float16)[:, None, None]
        blob[0:8, C_BIAS:C_W2] = bm.reshape(8, 128)
        blob[0:64, C_W2:C_TOT] = Wv[sl].T.astype(np.float16)
        blob[64:128, C_W2:C_TOT] = Wv[sl].T.astype(np.float16)
        in_maps.append({"blob": blob})

    nc = _get_nc()
    res = run_bass_kernel_spmd(nc, in_maps, core_ids=list(range(N_CORES)))
    LAST_RESULTS = res
    # device output is x-major fp16 [64, (c, y)]; upcast + transpose
    outs = [
        r["out"].astype(np.float32).reshape(64, 8, 64).transpose(1, 0, 2)
        for r in res.results
    ]
    out = np.concatenate(outs, axis=0)  # [64, 64, 64]
    return np.ascontiguousarray(out).reshape(1, 64, 64, 64)
